# revision 20
# baseline (speedup 1.0000x reference)
"""Trainium2 Bass kernel for 3D attention block (GroupNorm + 1x1x1-conv QKV +
4-head attention over N=4096 + out-projection + residual).

Sharding: 8 cores = 2 batches x 4 query-slices (sequence parallel).  Each core
receives its batch's x rolled by -n0 along the flattened spatial axis, so the
SPMD program always computes queries [0:1024] of its local view; softmax /
GroupNorm / the value contraction are invariant to the roll.  No cross-core
communication is needed; the host concatenates the 8 output slices.
"""

import sys

sys.path.insert(0, "/opt/trn_rl_repo")

import numpy as np

# Problem constants (hardcoded per harness contract).
B = 2
C = 128
D3 = (16, 16, 16)
N = 4096
NH = 4
HD = 32
GROUPS = 32
EPS = 1e-5
SCALE = float(HD) ** -0.5

NCORES = 8
NSPLIT = 4          # query-slices per batch
NSLICE = N // NSPLIT  # 1024 queries per core
NT = NSLICE // 512    # 512-wide query tiles per core
MC = N // 128         # 128-wide key chunks

_nc_cache = {}


def _build():
    import concourse.bass as bass
    import concourse.tile as tile
    from concourse import bacc, mybir

    f32 = mybir.dt.float32
    bf16 = mybir.dt.bfloat16
    AF = mybir.ActivationFunctionType
    ALU = mybir.AluOpType

    nc = bacc.Bacc()

    xr_d = nc.declare_dram_parameter("xr", [C, N], f32, isOutput=False)
    qkvwT_d = nc.declare_dram_parameter("qkvwT", [C, 3 * C], f32, isOutput=False)
    outwT_d = nc.declare_dram_parameter("outwT", [C, C], f32, isOutput=False)
    qb_d = nc.declare_dram_parameter("qb", [C, 1], f32, isOutput=False)
    kb_d = nc.declare_dram_parameter("kb", [C, 1], f32, isOutput=False)
    vb_d = nc.declare_dram_parameter("vb", [C, 1], f32, isOutput=False)
    ob_d = nc.declare_dram_parameter("ob", [C, 1], f32, isOutput=False)
    gnw_d = nc.declare_dram_parameter("gnw", [C, 1], f32, isOutput=False)
    gnb_d = nc.declare_dram_parameter("gnb", [C, 1], f32, isOutput=False)
    g2_d = nc.declare_dram_parameter("g2", [C, C], f32, isOutput=False)
    y_d = nc.declare_dram_parameter("y", [C, NSLICE], f32, isOutput=True)

    with tile.TileContext(nc) as tc:
        with tc.tile_pool(name="singles", bufs=1) as singles:
            # ---- input DMAs ----
            x_sb = singles.tile([C, N], f32)
            for t in range(8):
                nc.sync.dma_start(
                    out=x_sb[:, t * 512 : (t + 1) * 512],
                    in_=xr_d[:, t * 512 : (t + 1) * 512],
                )
            qkvwT_f = singles.tile([C, 3 * C], f32)
            nc.sync.dma_start(out=qkvwT_f, in_=qkvwT_d[:, :])
            outwT_f = singles.tile([C, C], f32)
            nc.sync.dma_start(out=outwT_f, in_=outwT_d[:, :])
            qb_sb = singles.tile([C, 1], f32)
            nc.sync.dma_start(out=qb_sb, in_=qb_d[:, :])
            kb_sb = singles.tile([C, 1], f32)
            nc.sync.dma_start(out=kb_sb, in_=kb_d[:, :])
            vb_sb = singles.tile([C, 1], f32)
            nc.sync.dma_start(out=vb_sb, in_=vb_d[:, :])
            ob_sb = singles.tile([C, 1], f32)
            nc.sync.dma_start(out=ob_sb, in_=ob_d[:, :])
            gnw_sb = singles.tile([C, 1], f32)
            nc.sync.dma_start(out=gnw_sb, in_=gnw_d[:, :])
            gnb_sb = singles.tile([C, 1], f32)
            nc.sync.dma_start(out=gnb_sb, in_=gnb_d[:, :])
            g2_sb = singles.tile([C, C], f32)
            nc.sync.dma_start(out=g2_sb, in_=g2_d[:, :])

            # bf16 weight casts
            qkvwT_b = singles.tile([C, 3 * C], bf16)
            nc.vector.tensor_copy(out=qkvwT_b, in_=qkvwT_f)
            outwT_b = singles.tile([C, C], bf16)
            nc.vector.tensor_copy(out=outwT_b, in_=outwT_f)
            vb_b = singles.tile([C, 1], bf16)
            nc.vector.tensor_copy(out=vb_b, in_=vb_sb)

            # ---- GroupNorm statistics ----
            stats = singles.tile([C, 8, 6], f32)
            for t in range(8):
                nc.vector.bn_stats(
                    out=stats[:, t, :], in_=x_sb[:, t * 512 : (t + 1) * 512]
                )
            mv = singles.tile([C, 2], f32)
            nc.vector.bn_aggr(out=mv, in_=stats)

            # m_ex = [mean_c, E_c[x^2]] per channel
            m_ex = singles.tile([C, 2], f32)
            nc.vector.tensor_copy(out=m_ex[:, 0:1], in_=mv[:, 0:1])
            msq = singles.tile([C, 1], f32)
            nc.vector.tensor_mul(out=msq, in0=mv[:, 0:1], in1=mv[:, 0:1])
            nc.vector.tensor_add(out=m_ex[:, 1:2], in0=mv[:, 1:2], in1=msq)

            xn_b = singles.tile([C, N], bf16)
            k_sb = singles.tile([C, N], bf16)
            q_sb = singles.tile([C, NSLICE], bf16)
            vt_sb = singles.tile([C, MC, NH, HD], bf16)
            ones_mat = singles.tile([C, HD], bf16)
            nc.vector.memset(ones_mat, 1.0)
            outb_eff = singles.tile([C, 1], f32)
            o4_sb = singles.tile([C, NSLICE], bf16)
            y_out = singles.tile([C, NSLICE], f32)

            # Route g2 through DVE so the (self-loading) fp32 matmul needs a
            # single semaphore wait — walrus allows only one on Matmult.
            g2_v = singles.tile([C, C], f32)
            nc.vector.tensor_copy(out=g2_v, in_=g2_sb)

            with tc.tile_pool(name="ppsum", bufs=2, space="PSUM") as ppool:
                # group-broadcast matmul: per-channel [mu_g, E_g[x^2]]
                gsp = ppool.tile([C, 2], f32, tag="gsp")
                nc.tensor.matmul(out=gsp, lhsT=g2_v, rhs=m_ex, start=True, stop=True)

                mu_g = singles.tile([C, 1], f32)
                nc.vector.tensor_copy(out=mu_g, in_=gsp[:, 0:1])
                musq = singles.tile([C, 1], f32)
                nc.vector.tensor_mul(out=musq, in0=mu_g, in1=mu_g)
                var_g = singles.tile([C, 1], f32)
                nc.vector.tensor_sub(out=var_g, in0=gsp[:, 1:2], in1=musq)

                eps_t = singles.tile([C, 1], f32)
                nc.vector.memset(eps_t, EPS)
                lnv = singles.tile([C, 1], f32)
                nc.scalar.activation(
                    out=lnv, in_=var_g, func=AF.Ln, bias=eps_t, scale=1.0
                )
                rstd = singles.tile([C, 1], f32)
                nc.scalar.activation(out=rstd, in_=lnv, func=AF.Exp, scale=-0.5)

                a_co = singles.tile([C, 1], f32)
                nc.vector.tensor_mul(out=a_co, in0=rstd, in1=gnw_sb)
                tmpb = singles.tile([C, 1], f32)
                nc.vector.tensor_mul(out=tmpb, in0=mu_g, in1=a_co)
                b_co = singles.tile([C, 1], f32)
                nc.vector.tensor_sub(out=b_co, in0=gnb_sb, in1=tmpb)

                # normalized input in bf16: xn = x*A + B
                for t in range(8):
                    nc.vector.tensor_scalar(
                        out=xn_b[:, t * 512 : (t + 1) * 512],
                        in0=x_sb[:, t * 512 : (t + 1) * 512],
                        scalar1=a_co,
                        scalar2=b_co,
                        op0=ALU.mult,
                        op1=ALU.add,
                    )

                # outb_eff = out_b + out_w @ v_bias   (folds v bias into epilogue)
                obe_p = ppool.tile([C, 1], f32, tag="gsp")
                nc.tensor.matmul(
                    out=obe_p, lhsT=outwT_b, rhs=vb_b, start=True, stop=True
                )
                nc.vector.tensor_add(out=outb_eff, in0=obe_p, in1=ob_sb)

                # ---- K / Q projections ----
                for t in range(8):
                    kp = ppool.tile([C, 512], f32, tag="kqp")
                    nc.tensor.matmul(
                        out=kp,
                        lhsT=qkvwT_b[:, C : 2 * C],
                        rhs=xn_b[:, t * 512 : (t + 1) * 512],
                        start=True,
                        stop=True,
                    )
                    nc.vector.tensor_scalar_add(
                        out=k_sb[:, t * 512 : (t + 1) * 512], in0=kp, scalar1=kb_sb
                    )
                for t in range(NT):
                    qp = ppool.tile([C, 512], f32, tag="kqp")
                    nc.tensor.matmul(
                        out=qp,
                        lhsT=qkvwT_b[:, 0:C],
                        rhs=xn_b[:, t * 512 : (t + 1) * 512],
                        start=True,
                        stop=True,
                    )
                    nc.vector.tensor_scalar_add(
                        out=q_sb[:, t * 512 : (t + 1) * 512], in0=qp, scalar1=qb_sb
                    )
                # ---- vT (value transposed, [m, head, d]) via xn-as-lhsT ----
                for mc in range(MC):
                    vp = ppool.tile([C, C], f32, tag="vtp")
                    nc.tensor.matmul(
                        out=vp,
                        lhsT=xn_b[:, mc * 128 : (mc + 1) * 128],
                        rhs=qkvwT_b[:, 2 * C : 3 * C],
                        start=True,
                        stop=True,
                    )
                    nc.scalar.activation(
                        out=vt_sb[:, mc, :, :],
                        in_=vp.rearrange("p (h d) -> p h d", h=NH),
                        func=AF.Copy,
                    )

            # ---- attention ----
            # PSUM: sA 2 + sB 2 + o (bufs=2 -> 2) + z/y shared (bufs=2 -> 2) = 8
            with tc.tile_pool(name="spsum", bufs=1, space="PSUM") as spool, \
                 tc.tile_pool(name="opsum", bufs=2, space="PSUM") as opool, \
                 tc.tile_pool(name="zpsum", bufs=2, space="PSUM") as zpool, \
                 tc.tile_pool(name="apool", bufs=2) as apool, \
                 tc.tile_pool(name="dpool", bufs=2) as dpool:
                for nt in range(NT):
                    qs = q_sb[:, nt * 512 : (nt + 1) * 512]
                    o4ps = opool.tile([C, 512], f32, tag="oacc")
                    z_ps = zpool.tile([C, 512], f32, tag="zy")
                    for mc in range(MC):
                        ks = k_sb[:, mc * 128 : (mc + 1) * 128]
                        # split into two half-chunks (heads 01 / 23) so the
                        # next chunk's S-matmuls overlap the other half's exp
                        sA = spool.tile([C, 2, 512], f32, tag="sA")
                        sB = spool.tile([C, 2, 512], f32, tag="sB")
                        for h in range(NH):
                            s_t = sA if h < 2 else sB
                            nc.tensor.matmul(
                                out=s_t[:, h % 2, :],
                                lhsT=ks[32 * h : 32 * h + 32, :],
                                rhs=qs[32 * h : 32 * h + 32, :],
                                start=True,
                                stop=True,
                                tile_position=(32 * h, 0),
                            )
                        eA = apool.tile([C, 2, 512], bf16, tag="eA")
                        eB = apool.tile([C, 2, 512], bf16, tag="eB")
                        nc.scalar.activation(out=eA, in_=sA, func=AF.Exp, scale=SCALE)
                        nc.scalar.activation(out=eB, in_=sB, func=AF.Exp, scale=SCALE)
                        first, last = (mc == 0), (mc == MC - 1)
                        for h in range(NH):
                            e_t = eA if h < 2 else eB
                            # col-tiled: 4 heads run concurrently in the PE
                            nc.tensor.matmul(
                                out=o4ps[32 * h : 32 * h + 32, :],
                                lhsT=vt_sb[:, mc, h, :],
                                rhs=e_t[:, h % 2, :],
                                start=first,
                                stop=last,
                                tile_position=(0, 32 * h),
                                skip_group_check=True,
                            )
                        for h in range(NH):
                            e_t = eA if h < 2 else eB
                            # all-ones lhsT: every output partition gets Z_h,
                            # i.e. the denominator arrives pre-broadcast
                            nc.tensor.matmul(
                                out=z_ps[32 * h : 32 * h + 32, :],
                                lhsT=ones_mat,
                                rhs=e_t[:, h % 2, :],
                                start=first,
                                stop=last,
                                tile_position=(0, 32 * h),
                                skip_group_check=True,
                            )
                    # ---- per-tile epilogue: 1/Z, out-projection, residual ----
                    r_rep = dpool.tile([C, 512], f32)
                    nc.vector.reciprocal_approx_fast(out=r_rep, in_=z_ps)
                    nc.vector.tensor_mul(
                        out=o4_sb[:, nt * 512 : (nt + 1) * 512], in0=o4ps, in1=r_rep
                    )

                    y_ps = zpool.tile([C, 512], f32, tag="zy")
                    nc.tensor.matmul(
                        out=y_ps,
                        lhsT=outwT_b,
                        rhs=o4_sb[:, nt * 512 : (nt + 1) * 512],
                        start=True,
                        stop=True,
                    )
                    nc.vector.scalar_tensor_tensor(
                        out=y_out[:, nt * 512 : (nt + 1) * 512],
                        in0=y_ps,
                        scalar=outb_eff,
                        in1=x_sb[:, nt * 512 : (nt + 1) * 512],
                        op0=ALU.add,
                        op1=ALU.add,
                    )
            nc.sync.dma_start(out=y_d[:, :], in_=y_out)
    nc.finalize()
    return nc


def get_nc():
    if "nc" not in _nc_cache:
        _nc_cache["nc"] = _build()
    return _nc_cache["nc"]


def make_in_maps(inputs):
    x = np.asarray(inputs["x"], dtype=np.float32)
    gn_w = np.asarray(inputs["gn_w"], dtype=np.float32)
    gn_b = np.asarray(inputs["gn_b"], dtype=np.float32)
    qkv_w = np.asarray(inputs["qkv_w"], dtype=np.float32)
    qkv_b = np.asarray(inputs["qkv_b"], dtype=np.float32)
    out_w = np.asarray(inputs["out_w"], dtype=np.float32)
    out_b = np.asarray(inputs["out_b"], dtype=np.float32)

    qkvwT = np.ascontiguousarray(qkv_w.T)                        # [C, 3C]
    outwT = np.ascontiguousarray(out_w.T)                        # [C, C]
    qb = np.ascontiguousarray(qkv_b[0:C].reshape(C, 1))
    kb = np.ascontiguousarray(qkv_b[C : 2 * C].reshape(C, 1))
    vb = np.ascontiguousarray(qkv_b[2 * C : 3 * C].reshape(C, 1))
    ob = np.ascontiguousarray(out_b.reshape(C, 1))
    gnw = np.ascontiguousarray(gn_w.reshape(C, 1))
    gnb = np.ascontiguousarray(gn_b.reshape(C, 1))
    cidx = np.arange(C)
    g2 = np.where((cidx[:, None] // (C // GROUPS)) == (cidx[None, :] // (C // GROUPS)),
                  np.float32(1.0 / (C // GROUPS)), np.float32(0.0)).astype(np.float32)

    xf = x.reshape(B, C, N)
    in_maps = []
    for core in range(NCORES):
        b, j = divmod(core, NSPLIT)
        n0 = j * NSLICE
        xr = np.ascontiguousarray(np.roll(xf[b], -n0, axis=1))
        in_maps.append(
            {
                "xr": xr,
                "qkvwT": qkvwT,
                "outwT": outwT,
                "qb": qb,
                "kb": kb,
                "vb": vb,
                "ob": ob,
                "gnw": gnw,
                "gnb": gnb,
                "g2": g2,
            }
        )
    return in_maps


def assemble(results):
    y = np.empty((B, C, N), dtype=np.float32)
    for core in range(NCORES):
        b, j = divmod(core, NSPLIT)
        y[b][:, j * NSLICE : (j + 1) * NSLICE] = results[core]["y"]
    return y.reshape(B, C, *D3)


def run(inputs, trace=False):
    from concourse.bass_utils import run_bass_kernel_spmd

    nc = get_nc()
    in_maps = make_in_maps(inputs)
    res = run_bass_kernel_spmd(
        nc, in_maps, core_ids=list(range(NCORES)), trace=trace
    )
    return assemble(res.results), res


def kernel(**inputs):
    out, _ = run(inputs, trace=False)
    return out


# revision 21
# speedup vs baseline: 1.6964x; 1.6964x over previous
"""Trainium2 Bass kernel for 3D attention block (GroupNorm + 1x1x1-conv QKV +
4-head attention over N=4096 + out-projection + residual).

Sharding: 8 cores = 2 batches x 4 query-slices (sequence parallel).  Each core
receives its batch's x rolled by -n0 along the flattened spatial axis, so the
SPMD program always computes queries [0:1024] of its local view; softmax /
GroupNorm / the value contraction are invariant to the roll.  No cross-core
communication is needed; the host concatenates the 8 output slices.
"""

import sys

sys.path.insert(0, "/opt/trn_rl_repo")

import numpy as np

# Problem constants (hardcoded per harness contract).
B = 2
C = 128
D3 = (16, 16, 16)
N = 4096
NH = 4
HD = 32
GROUPS = 32
EPS = 1e-5
SCALE = float(HD) ** -0.5

NCORES = 8
NSPLIT = 4          # query-slices per batch
NSLICE = N // NSPLIT  # 1024 queries per core
NT = NSLICE // 512    # 512-wide query tiles per core
MC = N // 128         # 128-wide key chunks

_nc_cache = {}


def _build():
    import concourse.bass as bass
    import concourse.tile as tile
    from concourse import bacc, mybir

    f32 = mybir.dt.float32
    bf16 = mybir.dt.bfloat16
    AF = mybir.ActivationFunctionType
    ALU = mybir.AluOpType

    nc = bacc.Bacc()

    xr_d = nc.declare_dram_parameter("xr", [C, N], f32, isOutput=False)
    qkvwT_d = nc.declare_dram_parameter("qkvwT", [C, 3 * C], f32, isOutput=False)
    outwT_d = nc.declare_dram_parameter("outwT", [C, C], f32, isOutput=False)
    qb_d = nc.declare_dram_parameter("qb", [C, 1], f32, isOutput=False)
    kb_d = nc.declare_dram_parameter("kb", [C, 1], f32, isOutput=False)
    vb_d = nc.declare_dram_parameter("vb", [C, 1], f32, isOutput=False)
    ob_d = nc.declare_dram_parameter("ob", [C, 1], f32, isOutput=False)
    gnw_d = nc.declare_dram_parameter("gnw", [C, 1], f32, isOutput=False)
    gnb_d = nc.declare_dram_parameter("gnb", [C, 1], f32, isOutput=False)
    g2_d = nc.declare_dram_parameter("g2", [C, C], f32, isOutput=False)
    y_d = nc.declare_dram_parameter("y", [C, NSLICE], f32, isOutput=True)

    with tile.TileContext(nc) as tc:
        with tc.tile_pool(name="singles", bufs=1) as singles:
            # ---- input DMAs ----
            x_sb = singles.tile([C, N], f32)
            for t in range(8):
                nc.sync.dma_start(
                    out=x_sb[:, t * 512 : (t + 1) * 512],
                    in_=xr_d[:, t * 512 : (t + 1) * 512],
                )
            qkvwT_f = singles.tile([C, 3 * C], f32)
            nc.sync.dma_start(out=qkvwT_f, in_=qkvwT_d[:, :])
            outwT_f = singles.tile([C, C], f32)
            nc.sync.dma_start(out=outwT_f, in_=outwT_d[:, :])
            qb_sb = singles.tile([C, 1], f32)
            nc.sync.dma_start(out=qb_sb, in_=qb_d[:, :])
            kb_sb = singles.tile([C, 1], f32)
            nc.sync.dma_start(out=kb_sb, in_=kb_d[:, :])
            vb_sb = singles.tile([C, 1], f32)
            nc.sync.dma_start(out=vb_sb, in_=vb_d[:, :])
            ob_sb = singles.tile([C, 1], f32)
            nc.sync.dma_start(out=ob_sb, in_=ob_d[:, :])
            gnw_sb = singles.tile([C, 1], f32)
            nc.sync.dma_start(out=gnw_sb, in_=gnw_d[:, :])
            gnb_sb = singles.tile([C, 1], f32)
            nc.sync.dma_start(out=gnb_sb, in_=gnb_d[:, :])
            g2_sb = singles.tile([C, C], f32)
            nc.sync.dma_start(out=g2_sb, in_=g2_d[:, :])

            # bf16 weight casts
            qkvwT_b = singles.tile([C, 3 * C], bf16)
            nc.vector.tensor_copy(out=qkvwT_b, in_=qkvwT_f)
            outwT_b = singles.tile([C, C], bf16)
            nc.vector.tensor_copy(out=outwT_b, in_=outwT_f)
            vb_b = singles.tile([C, 1], bf16)
            nc.vector.tensor_copy(out=vb_b, in_=vb_sb)

            # ---- GroupNorm statistics ----
            stats = singles.tile([C, 8, 6], f32)
            for t in range(8):
                nc.vector.bn_stats(
                    out=stats[:, t, :], in_=x_sb[:, t * 512 : (t + 1) * 512]
                )
            mv = singles.tile([C, 2], f32)
            nc.vector.bn_aggr(out=mv, in_=stats)

            # m_ex = [mean_c, E_c[x^2]] per channel
            m_ex = singles.tile([C, 2], f32)
            nc.vector.tensor_copy(out=m_ex[:, 0:1], in_=mv[:, 0:1])
            msq = singles.tile([C, 1], f32)
            nc.vector.tensor_mul(out=msq, in0=mv[:, 0:1], in1=mv[:, 0:1])
            nc.vector.tensor_add(out=m_ex[:, 1:2], in0=mv[:, 1:2], in1=msq)

            xn_b = singles.tile([C, N], bf16)
            k_sb = singles.tile([C, N], bf16)
            q_sb = singles.tile([C, NSLICE], bf16)
            vt_sb = singles.tile([C, MC, NH, HD], bf16)
            ones_mat = singles.tile([C, HD], bf16)
            nc.vector.memset(ones_mat, 1.0)
            outb_eff = singles.tile([C, 1], f32)
            o4_sb = singles.tile([C, NSLICE], bf16)
            y_out = singles.tile([C, NSLICE], f32)

            # Route g2 through DVE so the (self-loading) fp32 matmul needs a
            # single semaphore wait — walrus allows only one on Matmult.
            g2_v = singles.tile([C, C], f32)
            nc.vector.tensor_copy(out=g2_v, in_=g2_sb)

            with tc.tile_pool(name="ppsum", bufs=2, space="PSUM") as ppool:
                # group-broadcast matmul: per-channel [mu_g, E_g[x^2]]
                gsp = ppool.tile([C, 2], f32, tag="gsp")
                nc.tensor.matmul(out=gsp, lhsT=g2_v, rhs=m_ex, start=True, stop=True)

                mu_g = singles.tile([C, 1], f32)
                nc.vector.tensor_copy(out=mu_g, in_=gsp[:, 0:1])
                musq = singles.tile([C, 1], f32)
                nc.vector.tensor_mul(out=musq, in0=mu_g, in1=mu_g)
                var_g = singles.tile([C, 1], f32)
                nc.vector.tensor_sub(out=var_g, in0=gsp[:, 1:2], in1=musq)

                eps_t = singles.tile([C, 1], f32)
                nc.vector.memset(eps_t, EPS)
                lnv = singles.tile([C, 1], f32)
                nc.scalar.activation(
                    out=lnv, in_=var_g, func=AF.Ln, bias=eps_t, scale=1.0
                )
                rstd = singles.tile([C, 1], f32)
                nc.scalar.activation(out=rstd, in_=lnv, func=AF.Exp, scale=-0.5)

                a_co = singles.tile([C, 1], f32)
                nc.vector.tensor_mul(out=a_co, in0=rstd, in1=gnw_sb)
                tmpb = singles.tile([C, 1], f32)
                nc.vector.tensor_mul(out=tmpb, in0=mu_g, in1=a_co)
                b_co = singles.tile([C, 1], f32)
                nc.vector.tensor_sub(out=b_co, in0=gnb_sb, in1=tmpb)

                # normalized input in bf16: xn = x*A + B
                for t in range(8):
                    nc.vector.tensor_scalar(
                        out=xn_b[:, t * 512 : (t + 1) * 512],
                        in0=x_sb[:, t * 512 : (t + 1) * 512],
                        scalar1=a_co,
                        scalar2=b_co,
                        op0=ALU.mult,
                        op1=ALU.add,
                    )

                # outb_eff = out_b + out_w @ v_bias   (folds v bias into epilogue)
                obe_p = ppool.tile([C, 1], f32, tag="gsp")
                nc.tensor.matmul(
                    out=obe_p, lhsT=outwT_b, rhs=vb_b, start=True, stop=True
                )
                nc.vector.tensor_add(out=outb_eff, in0=obe_p, in1=ob_sb)

                # ---- K / Q projections ----
                for t in range(8):
                    kp = ppool.tile([C, 512], f32, tag="kqp")
                    nc.tensor.matmul(
                        out=kp,
                        lhsT=qkvwT_b[:, C : 2 * C],
                        rhs=xn_b[:, t * 512 : (t + 1) * 512],
                        start=True,
                        stop=True,
                    )
                    nc.vector.tensor_scalar_add(
                        out=k_sb[:, t * 512 : (t + 1) * 512], in0=kp, scalar1=kb_sb
                    )
                for t in range(NT):
                    qp = ppool.tile([C, 512], f32, tag="kqp")
                    nc.tensor.matmul(
                        out=qp,
                        lhsT=qkvwT_b[:, 0:C],
                        rhs=xn_b[:, t * 512 : (t + 1) * 512],
                        start=True,
                        stop=True,
                    )
                    nc.vector.tensor_scalar_add(
                        out=q_sb[:, t * 512 : (t + 1) * 512], in0=qp, scalar1=qb_sb
                    )
                # ---- vT (value transposed, [m, head, d]) via xn-as-lhsT ----
                for mc in range(MC):
                    vp = ppool.tile([C, C], f32, tag="vtp")
                    nc.tensor.matmul(
                        out=vp,
                        lhsT=xn_b[:, mc * 128 : (mc + 1) * 128],
                        rhs=qkvwT_b[:, 2 * C : 3 * C],
                        start=True,
                        stop=True,
                    )
                    nc.scalar.activation(
                        out=vt_sb[:, mc, :, :],
                        in_=vp.rearrange("p (h d) -> p h d", h=NH),
                        func=AF.Copy,
                    )

            # ---- attention ----
            # PSUM: sA 2 + sB 2 + o (bufs=2 -> 2) + z/y shared (bufs=2 -> 2) = 8
            with tc.tile_pool(name="spsum", bufs=1, space="PSUM") as spool, \
                 tc.tile_pool(name="opsum", bufs=2, space="PSUM") as opool, \
                 tc.tile_pool(name="zpsum", bufs=2, space="PSUM") as zpool, \
                 tc.tile_pool(name="apool", bufs=2) as apool, \
                 tc.tile_pool(name="dpool", bufs=2) as dpool:
                for nt in range(NT):
                    qs = q_sb[:, nt * 512 : (nt + 1) * 512]
                    o4ps = opool.tile([C, 512], f32, tag="oacc")
                    z_ps = zpool.tile([C, 512], f32, tag="zy")

                    def emit_oz(mc, eA, eB):
                        first, last = (mc == 0), (mc == MC - 1)
                        for h in range(NH):
                            e_t = eA if h < 2 else eB
                            # col-tiled: 4 heads run concurrently in the PE
                            nc.tensor.matmul(
                                out=o4ps[32 * h : 32 * h + 32, :],
                                lhsT=vt_sb[:, mc, h, :],
                                rhs=e_t[:, h % 2, :],
                                start=first,
                                stop=last,
                                tile_position=(0, 32 * h),
                                skip_group_check=True,
                            )
                        for h in range(NH):
                            e_t = eA if h < 2 else eB
                            # all-ones lhsT: every output partition gets Z_h,
                            # i.e. the denominator arrives pre-broadcast
                            nc.tensor.matmul(
                                out=z_ps[32 * h : 32 * h + 32, :],
                                lhsT=ones_mat,
                                rhs=e_t[:, h % 2, :],
                                start=first,
                                stop=last,
                                tile_position=(0, 32 * h),
                                skip_group_check=True,
                            )

                    # software pipeline: emit chunk mc's S-matmuls and exps,
                    # then chunk mc-1's o/z matmuls, so the PE stream runs the
                    # next S group while ScalarE is still exponentiating.
                    pending = None
                    for mc in range(MC):
                        ks = k_sb[:, mc * 128 : (mc + 1) * 128]
                        # split into two half-chunks (heads 01 / 23) so the
                        # next chunk's S-matmuls overlap the other half's exp
                        sA = spool.tile([C, 2, 512], f32, tag="sA")
                        sB = spool.tile([C, 2, 512], f32, tag="sB")
                        for h in range(NH):
                            s_t = sA if h < 2 else sB
                            nc.tensor.matmul(
                                out=s_t[:, h % 2, :],
                                lhsT=ks[32 * h : 32 * h + 32, :],
                                rhs=qs[32 * h : 32 * h + 32, :],
                                start=True,
                                stop=True,
                                tile_position=(32 * h, 0),
                            )
                        eA = apool.tile([C, 2, 512], bf16, tag="eA")
                        eB = apool.tile([C, 2, 512], bf16, tag="eB")
                        nc.scalar.activation(out=eA, in_=sA, func=AF.Exp, scale=SCALE)
                        nc.scalar.activation(out=eB, in_=sB, func=AF.Exp, scale=SCALE)
                        if pending is not None:
                            emit_oz(*pending)
                        pending = (mc, eA, eB)
                    emit_oz(*pending)
                    # ---- per-tile epilogue: 1/Z, out-projection, residual ----
                    r_rep = dpool.tile([C, 512], f32)
                    nc.vector.reciprocal_approx_fast(out=r_rep, in_=z_ps)
                    nc.vector.tensor_mul(
                        out=o4_sb[:, nt * 512 : (nt + 1) * 512], in0=o4ps, in1=r_rep
                    )

                    y_ps = zpool.tile([C, 512], f32, tag="zy")
                    nc.tensor.matmul(
                        out=y_ps,
                        lhsT=outwT_b,
                        rhs=o4_sb[:, nt * 512 : (nt + 1) * 512],
                        start=True,
                        stop=True,
                    )
                    nc.vector.scalar_tensor_tensor(
                        out=y_out[:, nt * 512 : (nt + 1) * 512],
                        in0=y_ps,
                        scalar=outb_eff,
                        in1=x_sb[:, nt * 512 : (nt + 1) * 512],
                        op0=ALU.add,
                        op1=ALU.add,
                    )
            nc.sync.dma_start(out=y_d[:, :], in_=y_out)
    nc.finalize()
    return nc


def get_nc():
    if "nc" not in _nc_cache:
        _nc_cache["nc"] = _build()
    return _nc_cache["nc"]


def make_in_maps(inputs):
    x = np.asarray(inputs["x"], dtype=np.float32)
    gn_w = np.asarray(inputs["gn_w"], dtype=np.float32)
    gn_b = np.asarray(inputs["gn_b"], dtype=np.float32)
    qkv_w = np.asarray(inputs["qkv_w"], dtype=np.float32)
    qkv_b = np.asarray(inputs["qkv_b"], dtype=np.float32)
    out_w = np.asarray(inputs["out_w"], dtype=np.float32)
    out_b = np.asarray(inputs["out_b"], dtype=np.float32)

    qkvwT = np.ascontiguousarray(qkv_w.T)                        # [C, 3C]
    outwT = np.ascontiguousarray(out_w.T)                        # [C, C]
    qb = np.ascontiguousarray(qkv_b[0:C].reshape(C, 1))
    kb = np.ascontiguousarray(qkv_b[C : 2 * C].reshape(C, 1))
    vb = np.ascontiguousarray(qkv_b[2 * C : 3 * C].reshape(C, 1))
    ob = np.ascontiguousarray(out_b.reshape(C, 1))
    gnw = np.ascontiguousarray(gn_w.reshape(C, 1))
    gnb = np.ascontiguousarray(gn_b.reshape(C, 1))
    cidx = np.arange(C)
    g2 = np.where((cidx[:, None] // (C // GROUPS)) == (cidx[None, :] // (C // GROUPS)),
                  np.float32(1.0 / (C // GROUPS)), np.float32(0.0)).astype(np.float32)

    xf = x.reshape(B, C, N)
    in_maps = []
    for core in range(NCORES):
        b, j = divmod(core, NSPLIT)
        n0 = j * NSLICE
        xr = np.ascontiguousarray(np.roll(xf[b], -n0, axis=1))
        in_maps.append(
            {
                "xr": xr,
                "qkvwT": qkvwT,
                "outwT": outwT,
                "qb": qb,
                "kb": kb,
                "vb": vb,
                "ob": ob,
                "gnw": gnw,
                "gnb": gnb,
                "g2": g2,
            }
        )
    return in_maps


def assemble(results):
    y = np.empty((B, C, N), dtype=np.float32)
    for core in range(NCORES):
        b, j = divmod(core, NSPLIT)
        y[b][:, j * NSLICE : (j + 1) * NSLICE] = results[core]["y"]
    return y.reshape(B, C, *D3)


def run(inputs, trace=False):
    from concourse.bass_utils import run_bass_kernel_spmd

    nc = get_nc()
    in_maps = make_in_maps(inputs)
    res = run_bass_kernel_spmd(
        nc, in_maps, core_ids=list(range(NCORES)), trace=trace
    )
    return assemble(res.results), res


def kernel(**inputs):
    out, _ = run(inputs, trace=False)
    return out


# revision 24
# speedup vs baseline: 2.0540x; 1.2108x over previous
"""Trainium2 Bass kernel for 3D attention block (GroupNorm + 1x1x1-conv QKV +
4-head attention over N=4096 + out-projection + residual).

Sharding: 8 cores = 2 batches x 4 query-slices (sequence parallel).  Each core
receives its batch's x rolled by -n0 along the flattened spatial axis, so the
SPMD program always computes queries [0:1024] of its local view; softmax /
GroupNorm / the value contraction are invariant to the roll.  No cross-core
communication is needed; the host concatenates the 8 output slices.
"""

import sys

sys.path.insert(0, "/opt/trn_rl_repo")

import numpy as np

# Problem constants (hardcoded per harness contract).
B = 2
C = 128
D3 = (16, 16, 16)
N = 4096
NH = 4
HD = 32
GROUPS = 32
EPS = 1e-5
SCALE = float(HD) ** -0.5

NCORES = 8
NSPLIT = 4          # query-slices per batch
NSLICE = N // NSPLIT  # 1024 queries per core
NT = NSLICE // 512    # 512-wide query tiles per core
MC = N // 128         # 128-wide key chunks
WPC = 5 * C + 6       # packed weight blob columns

_nc_cache = {}


def _build():
    import concourse.bass as bass
    import concourse.tile as tile
    from concourse import bacc, mybir

    f32 = mybir.dt.float32
    bf16 = mybir.dt.bfloat16
    AF = mybir.ActivationFunctionType
    ALU = mybir.AluOpType

    nc = bacc.Bacc()

    xr_d = nc.declare_dram_parameter("xr", [C, N], f32, isOutput=False)
    wp_d = nc.declare_dram_parameter("wpack", [C, WPC], f32, isOutput=False)
    y_d = nc.declare_dram_parameter("y", [C, NSLICE], f32, isOutput=True)

    with tile.TileContext(nc) as tc:
        with tc.tile_pool(name="singles", bufs=1) as singles:
            # ---- input DMAs: x in 4 chunks on sync, weights as one packed
            # blob on the gpsimd queue (parallel issue) ----
            x_sb = singles.tile([C, N], f32)
            for t in range(4):
                nc.sync.dma_start(
                    out=x_sb[:, t * 1024 : (t + 1) * 1024],
                    in_=xr_d[:, t * 1024 : (t + 1) * 1024],
                )
            wpack = singles.tile([C, WPC], f32)
            nc.gpsimd.dma_start(out=wpack, in_=wp_d[:, :])
            qkvw_f = wpack[:, 0 : 3 * C]
            outw_f = wpack[:, 3 * C : 4 * C]
            g2_f = wpack[:, 4 * C : 5 * C]
            qb_ap = wpack[:, 5 * C + 0 : 5 * C + 1]
            kb_ap = wpack[:, 5 * C + 1 : 5 * C + 2]
            vb_ap = wpack[:, 5 * C + 2 : 5 * C + 3]
            ob_ap = wpack[:, 5 * C + 3 : 5 * C + 4]
            gnw_ap = wpack[:, 5 * C + 4 : 5 * C + 5]
            gnb_ap = wpack[:, 5 * C + 5 : 5 * C + 6]

            # force both activation table sets to load early (overlaps DMA)
            eps_t = singles.tile([C, 1], f32)
            nc.vector.memset(eps_t, EPS)
            scr0 = singles.tile([C, 1], f32)
            nc.scalar.activation(out=scr0, in_=eps_t, func=AF.Ln)
            scr1 = singles.tile([C, 1], f32)
            nc.scalar.activation(out=scr1, in_=eps_t, func=AF.Exp)

            # ---- GroupNorm statistics + x cast, pipelined per 512-chunk ----
            stats = singles.tile([C, 8, 6], f32)
            xb = singles.tile([C, N], bf16)
            for t in range(8):
                nc.vector.bn_stats(
                    out=stats[:, t, :], in_=x_sb[:, t * 512 : (t + 1) * 512]
                )
                nc.vector.tensor_copy(
                    out=xb[:, t * 512 : (t + 1) * 512],
                    in_=x_sb[:, t * 512 : (t + 1) * 512],
                )
            mv = singles.tile([C, 2], f32)
            nc.vector.bn_aggr(out=mv, in_=stats)

            # m_ex = [mean_c, E_c[x^2]] per channel
            m_ex = singles.tile([C, 2], f32)
            nc.vector.tensor_copy(out=m_ex[:, 0:1], in_=mv[:, 0:1])
            msq = singles.tile([C, 1], f32)
            nc.vector.tensor_mul(out=msq, in0=mv[:, 0:1], in1=mv[:, 0:1])
            nc.vector.tensor_add(out=m_ex[:, 1:2], in0=mv[:, 1:2], in1=msq)

            g2_v = singles.tile([C, C], f32)
            nc.vector.tensor_copy(out=g2_v, in_=g2_f)
            outwT_b = singles.tile([C, C], bf16)
            nc.vector.tensor_copy(out=outwT_b, in_=outw_f)

            k_sb = singles.tile([C, N], bf16)
            q_sb = singles.tile([C, NSLICE], bf16)
            vt_sb = singles.tile([C, MC, NH, HD], bf16)
            ones_mat = singles.tile([C, HD], bf16)
            nc.vector.memset(ones_mat, 1.0)
            outb_eff = singles.tile([C, 1], f32)
            o4_sb = singles.tile([C, NSLICE], bf16)
            y_out = singles.tile([C, NSLICE], f32)

            with tc.tile_pool(name="ppsum", bufs=2, space="PSUM") as ppool:
                # group-broadcast matmul: per-channel [mu_g, E_g[x^2]]
                gsp = ppool.tile([C, 2], f32, tag="gsp")
                nc.tensor.matmul(out=gsp, lhsT=g2_v, rhs=m_ex, start=True, stop=True)

                mu_g = singles.tile([C, 1], f32)
                nc.vector.tensor_copy(out=mu_g, in_=gsp[:, 0:1])
                musq = singles.tile([C, 1], f32)
                nc.vector.tensor_mul(out=musq, in0=mu_g, in1=mu_g)
                var_g = singles.tile([C, 1], f32)
                nc.vector.tensor_sub(out=var_g, in0=gsp[:, 1:2], in1=musq)

                lnv = singles.tile([C, 1], f32)
                nc.scalar.activation(
                    out=lnv, in_=var_g, func=AF.Ln, bias=eps_t, scale=1.0
                )
                rstd = singles.tile([C, 1], f32)
                nc.scalar.activation(out=rstd, in_=lnv, func=AF.Exp, scale=-0.5)

                a_co = singles.tile([C, 1], f32)
                nc.vector.tensor_mul(out=a_co, in0=rstd, in1=gnw_ap)
                tmpb = singles.tile([C, 1], f32)
                nc.vector.tensor_mul(out=tmpb, in0=mu_g, in1=a_co)
                b_co = singles.tile([C, 1], f32)
                nc.vector.tensor_sub(out=b_co, in0=gnb_ap, in1=tmpb)
                bco_b = singles.tile([C, 1], bf16)
                nc.vector.tensor_copy(out=bco_b, in_=b_co)

                # GroupNorm folded into the projection weights:
                # qkv(xn) = (W*diag(A)) x + (W @ B + qkv_b)
                qkvw_s = singles.tile([C, 3 * C], bf16)
                nc.vector.tensor_scalar_mul(out=qkvw_s, in0=qkvw_f, scalar1=a_co)

                # effective biases
                bqp = ppool.tile([C, 3], f32, tag="gsp")
                for s in range(3):
                    nc.tensor.matmul(
                        out=bqp[:, s : s + 1],
                        lhsT=qkvw_s[:, s * C : (s + 1) * C],
                        rhs=bco_b,
                        start=True,
                        stop=True,
                        skip_group_check=True,
                    )
                qb_eff = singles.tile([C, 1], f32)
                nc.vector.tensor_add(out=qb_eff, in0=bqp[:, 0:1], in1=qb_ap)
                kb_eff = singles.tile([C, 1], f32)
                nc.vector.tensor_add(out=kb_eff, in0=bqp[:, 1:2], in1=kb_ap)
                vvb = singles.tile([C, 1], f32)
                nc.vector.tensor_add(out=vvb, in0=bqp[:, 2:3], in1=vb_ap)
                vvb_b = singles.tile([C, 1], bf16)
                nc.vector.tensor_copy(out=vvb_b, in_=vvb)
                obe_p = ppool.tile([C, 1], f32, tag="obe")
                nc.tensor.matmul(
                    out=obe_p, lhsT=outwT_b, rhs=vvb_b, start=True, stop=True
                )
                nc.vector.tensor_add(out=outb_eff, in0=obe_p, in1=ob_ap)

                # ---- Q / K projections ----
                for t in range(NT):
                    qp = ppool.tile([C, 512], f32, tag="kqp")
                    nc.tensor.matmul(
                        out=qp,
                        lhsT=qkvw_s[:, 0:C],
                        rhs=xb[:, t * 512 : (t + 1) * 512],
                        start=True,
                        stop=True,
                    )
                    nc.vector.tensor_scalar_add(
                        out=q_sb[:, t * 512 : (t + 1) * 512], in0=qp, scalar1=qb_eff
                    )
                for t in range(8):
                    kp = ppool.tile([C, 512], f32, tag="kqp")
                    nc.tensor.matmul(
                        out=kp,
                        lhsT=qkvw_s[:, C : 2 * C],
                        rhs=xb[:, t * 512 : (t + 1) * 512],
                        start=True,
                        stop=True,
                    )
                    nc.vector.tensor_scalar_add(
                        out=k_sb[:, t * 512 : (t + 1) * 512], in0=kp, scalar1=kb_eff
                    )
                # ---- vT (value transposed, [m, head, d]) via xb-as-lhsT ----
                # 4 chunks per PSUM bank, one batched ScalarE drain per bank
                for g in range(8):
                    vp = ppool.tile([C, 4, C], f32, tag="vtp")
                    for j in range(4):
                        mc = 4 * g + j
                        nc.tensor.matmul(
                            out=vp[:, j, :],
                            lhsT=xb[:, mc * 128 : (mc + 1) * 128],
                            rhs=qkvw_s[:, 2 * C : 3 * C],
                            start=True,
                            stop=True,
                            skip_group_check=True,
                        )
                    nc.scalar.activation(
                        out=vt_sb[:, 4 * g : 4 * g + 4, :, :],
                        in_=vp.rearrange("p c (h d) -> p c h d", h=NH),
                        func=AF.Copy,
                    )

            # ---- attention ----
            # PSUM: sA 2 + sB 2 + o (bufs=2 -> 2) + z/y shared (bufs=2 -> 2) = 8
            with tc.tile_pool(name="spsum", bufs=1, space="PSUM") as spool, \
                 tc.tile_pool(name="opsum", bufs=2, space="PSUM") as opool, \
                 tc.tile_pool(name="zpsum", bufs=2, space="PSUM") as zpool, \
                 tc.tile_pool(name="apool", bufs=2) as apool, \
                 tc.tile_pool(name="dpool", bufs=2) as dpool:
                for nt in range(NT):
                    qs = q_sb[:, nt * 512 : (nt + 1) * 512]
                    o4ps = opool.tile([C, 512], f32, tag="oacc")
                    z_ps = zpool.tile([C, 512], f32, tag="zy")

                    def emit_oz(mc, eA, eB):
                        first, last = (mc == 0), (mc == MC - 1)
                        for h in range(NH):
                            e_t = eA if h < 2 else eB
                            # col-tiled: 4 heads run concurrently in the PE
                            nc.tensor.matmul(
                                out=o4ps[32 * h : 32 * h + 32, :],
                                lhsT=vt_sb[:, mc, h, :],
                                rhs=e_t[:, h % 2, :],
                                start=first,
                                stop=last,
                                tile_position=(0, 32 * h),
                                skip_group_check=True,
                            )
                        for h in range(NH):
                            e_t = eA if h < 2 else eB
                            # all-ones lhsT: every output partition gets Z_h,
                            # i.e. the denominator arrives pre-broadcast
                            nc.tensor.matmul(
                                out=z_ps[32 * h : 32 * h + 32, :],
                                lhsT=ones_mat,
                                rhs=e_t[:, h % 2, :],
                                start=first,
                                stop=last,
                                tile_position=(0, 32 * h),
                                skip_group_check=True,
                            )

                    # software pipeline: emit chunk mc's S-matmuls and exps,
                    # then chunk mc-1's o/z matmuls, so the PE stream runs the
                    # next S group while ScalarE is still exponentiating.
                    pending = None
                    for mc in range(MC):
                        ks = k_sb[:, mc * 128 : (mc + 1) * 128]
                        # split into two half-chunks (heads 01 / 23) so the
                        # next chunk's S-matmuls overlap the other half's exp
                        sA = spool.tile([C, 2, 512], f32, tag="sA")
                        sB = spool.tile([C, 2, 512], f32, tag="sB")
                        for h in range(NH):
                            s_t = sA if h < 2 else sB
                            nc.tensor.matmul(
                                out=s_t[:, h % 2, :],
                                lhsT=ks[32 * h : 32 * h + 32, :],
                                rhs=qs[32 * h : 32 * h + 32, :],
                                start=True,
                                stop=True,
                                tile_position=(32 * h, 0),
                            )
                        eA = apool.tile([C, 2, 512], bf16, tag="eA")
                        eB = apool.tile([C, 2, 512], bf16, tag="eB")
                        nc.scalar.activation(out=eA, in_=sA, func=AF.Exp, scale=SCALE)
                        nc.scalar.activation(out=eB, in_=sB, func=AF.Exp, scale=SCALE)
                        if pending is not None:
                            emit_oz(*pending)
                        pending = (mc, eA, eB)
                    emit_oz(*pending)
                    # ---- per-tile epilogue: 1/Z, out-projection, residual ----
                    r_rep = dpool.tile([C, 512], f32)
                    nc.vector.reciprocal_approx_fast(out=r_rep, in_=z_ps)
                    nc.vector.tensor_mul(
                        out=o4_sb[:, nt * 512 : (nt + 1) * 512], in0=o4ps, in1=r_rep
                    )

                    y_ps = zpool.tile([C, 512], f32, tag="zy")
                    nc.tensor.matmul(
                        out=y_ps,
                        lhsT=outwT_b,
                        rhs=o4_sb[:, nt * 512 : (nt + 1) * 512],
                        start=True,
                        stop=True,
                    )
                    nc.vector.scalar_tensor_tensor(
                        out=y_out[:, nt * 512 : (nt + 1) * 512],
                        in0=y_ps,
                        scalar=outb_eff,
                        in1=x_sb[:, nt * 512 : (nt + 1) * 512],
                        op0=ALU.add,
                        op1=ALU.add,
                    )
                    nc.sync.dma_start(
                        out=y_d[:, nt * 512 : (nt + 1) * 512],
                        in_=y_out[:, nt * 512 : (nt + 1) * 512],
                    )
    nc.finalize()
    return nc


def get_nc():
    if "nc" not in _nc_cache:
        _nc_cache["nc"] = _build()
    return _nc_cache["nc"]


def make_in_maps(inputs):
    x = np.asarray(inputs["x"], dtype=np.float32)
    gn_w = np.asarray(inputs["gn_w"], dtype=np.float32)
    gn_b = np.asarray(inputs["gn_b"], dtype=np.float32)
    qkv_w = np.asarray(inputs["qkv_w"], dtype=np.float32)
    qkv_b = np.asarray(inputs["qkv_b"], dtype=np.float32)
    out_w = np.asarray(inputs["out_w"], dtype=np.float32)
    out_b = np.asarray(inputs["out_b"], dtype=np.float32)

    cidx = np.arange(C)
    g2 = np.where((cidx[:, None] // (C // GROUPS)) == (cidx[None, :] // (C // GROUPS)),
                  np.float32(1.0 / (C // GROUPS)), np.float32(0.0)).astype(np.float32)
    wpack = np.empty((C, WPC), dtype=np.float32)
    wpack[:, 0 : 3 * C] = qkv_w.T
    wpack[:, 3 * C : 4 * C] = out_w.T
    wpack[:, 4 * C : 5 * C] = g2
    wpack[:, 5 * C + 0] = qkv_b[0:C]
    wpack[:, 5 * C + 1] = qkv_b[C : 2 * C]
    wpack[:, 5 * C + 2] = qkv_b[2 * C : 3 * C]
    wpack[:, 5 * C + 3] = out_b
    wpack[:, 5 * C + 4] = gn_w
    wpack[:, 5 * C + 5] = gn_b

    xf = x.reshape(B, C, N)
    in_maps = []
    for core in range(NCORES):
        b, j = divmod(core, NSPLIT)
        n0 = j * NSLICE
        xr = np.ascontiguousarray(np.roll(xf[b], -n0, axis=1))
        in_maps.append(
            {
                "xr": xr,
                "wpack": wpack,
            }
        )
    return in_maps


def assemble(results):
    y = np.empty((B, C, N), dtype=np.float32)
    for core in range(NCORES):
        b, j = divmod(core, NSPLIT)
        y[b][:, j * NSLICE : (j + 1) * NSLICE] = results[core]["y"]
    return y.reshape(B, C, *D3)


def run(inputs, trace=False):
    from concourse.bass_utils import run_bass_kernel_spmd

    nc = get_nc()
    in_maps = make_in_maps(inputs)
    res = run_bass_kernel_spmd(
        nc, in_maps, core_ids=list(range(NCORES)), trace=trace
    )
    return assemble(res.results), res


def kernel(**inputs):
    out, _ = run(inputs, trace=False)
    return out
